# revision 3
# baseline (speedup 1.0000x reference)
"""GRU kernel for 8 NeuronCores — fully on-device.

Sharding: data-parallel over batch (8 batch rows per core), per the
sharding hint. Each core runs the whole model for its batch shard:

  Phase A: input projections  gates_x = x @ [Wxz|Wxr|Wxh]^T + biases
           (bf16 matmuls, fp32 psum, evacuated to DRAM in the
           "stacked" layout the recurrence consumes).
  Phase B: 512-step recurrence. The per-step GEMM h @ Wh*^T runs with
           the transposed hidden state as the 128x8 stationary operand
           and the weights as the bf16 moving operand, in 128x32
           column-tiled mode: 4 concurrent column-tiles each compute a
           256-wide chunk of the hidden dimension, so the 3*H*H weight
           stream costs ~6k cycles/step instead of ~24k.
           gates_x is folded into PSUM with shifted-identity matmuls,
           sigmoids/tanh read PSUM directly on ScalarE, the h-update
           runs on VectorE, and the new h is re-transposed with two PE
           transposes into an 8-slot static history buffer that serves
           both as next-step stationary operand and as the archive-DMA
           source (matmul stationaries cannot take register offsets).
  Phase C: output projection from the DRAM h^T archive.

Layouts
  stacked [128, n]: partition 32g+b (g=hidden col-group 0..3, b=batch
  row 0..7) holds hidden cols [256g, 256g+n) for batch row b.
  gate-col layout (3072): cols g*512+gate*256+j for z/r (gate 0/1),
  2048+g*256+j for the candidate gate.
  hist [128, 512] bf16: col s*64 + k*8 + b = h^T[128k+p, b] at slot
  s = t%8.
  gzr/gh DRAM: [(S/8)*128, 8*n] — row blk*128+p, col s*n+j.
  hsd archive [128, 8, S*8]: (p, k, t*8+b).
"""
import time

import numpy as np
import ml_dtypes

import concourse.bass as bass
import concourse.tile as tile
from concourse import mybir
from concourse.bass import ds

SEQ, B, I, H, O = 512, 64, 512, 1024, 512
NCORES = 8
BL = B // NCORES          # 8 batch rows per core
NG = 4                    # column groups
GC = H // NG              # 256 hidden cols per group
TUNROLL = 16
KH = H // 128             # 8 contraction chunks for H
KI = I // 128             # 4 contraction chunks for I

bf16 = mybir.dt.bfloat16
f32 = mybir.dt.float32
AF = mybir.ActivationFunctionType
ALU = mybir.AluOpType

BF = ml_dtypes.bfloat16


def _legalize_multi_waits(nc):
    """This container's walrus encodes at most ONE sync-wait per TPB
    instruction and dies with "Too many sync wait commands" on anything
    carrying more (e.g. the Tile end-of-kernel drain). Hoist all-but-one
    wait of every instruction onto standalone sequencer EventSemaphore
    instructions placed immediately before it in the same engine stream."""
    n_multi_update = 0
    uid = [0]
    for fn in nc.m.functions:
        for bb in fn.blocks:
            insts = list(bb.instructions)
            out = []
            changed = False
            for inst in insts:
                si = inst.sync_info
                if si is not None and len(si.on_wait) > 1:
                    waits = list(si.on_wait)
                    for w in waits[:-1]:
                        uid[0] += 1
                        ev = mybir.InstEventSemaphore(
                            name=f"legalw-{uid[0]}", ins=[], outs=[]
                        )
                        ev.engine = inst.engine
                        ev.sync_info = mybir.SyncInfo(on_wait=[w], on_update=[])
                        out.append(ev)
                    inst.sync_info = mybir.SyncInfo(
                        on_wait=[waits[-1]], on_update=list(si.on_update)
                    )
                    changed = True
                if si is not None and len(si.on_update) > 1:
                    n_multi_update += 1
                out.append(inst)
            if changed:
                bb.instructions = out
    assert n_multi_update == 0


def build(n_steps=SEQ, legalize=True):
    n_iter = n_steps // TUNROLL
    assert n_iter * TUNROLL == n_steps
    nblk = n_steps // 8
    nc = bass.Bass()

    xT_d = nc.dram_tensor("xT", [I, n_steps * BL], bf16, kind="ExternalInput")
    wx_d = nc.dram_tensor("wxT", [I, 3 * H], bf16, kind="ExternalInput")
    wh_d = nc.dram_tensor("whT", [H, 3 * H], bf16, kind="ExternalInput")
    why_d = nc.dram_tensor("whyT", [H, O], bf16, kind="ExternalInput")
    bx_d = nc.dram_tensor("bxrow", [1, 3 * H], f32, kind="ExternalInput")
    bhy_d = nc.dram_tensor("bhyrow", [1, O], f32, kind="ExternalInput")
    bhh_d = nc.dram_tensor("bhhbc", [128, GC], bf16, kind="ExternalInput")
    id_d = nc.dram_tensor("ident", [128, 128], bf16, kind="ExternalInput")
    idf_d = nc.dram_tensor("identf", [128, 128], f32, kind="ExternalInput")
    ones_d = nc.dram_tensor("onesrow", [1, 128], f32, kind="ExternalInput")
    out_d = nc.dram_tensor("out", [n_steps, BL, O], f32, kind="ExternalOutput")

    with tile.TileContext(nc) as tc:
        with (
            tc.tile_pool(name="dram", bufs=1, space="DRAM") as dpool,
            tc.tile_pool(name="persist", bufs=1) as pp,
            tc.tile_pool(name="gin", bufs=1) as gpool,
            tc.tile_pool(name="tail", bufs=2) as tp,
            tc.tile_pool(name="evac", bufs=3) as ep,
            tc.tile_pool(name="cstat", bufs=3) as cpool,
            tc.tile_pool(name="pszr", bufs=2, space="PSUM") as pszr,
            tc.tile_pool(name="psh", bufs=2, space="PSUM") as psh,
            tc.tile_pool(name="pstr", bufs=2, space="PSUM") as pstr,
            tc.tile_pool(name="psio", bufs=2, space="PSUM") as psio,
        ):
            gzr_d = dpool.tile([nblk * 128, 8 * 512], bf16)
            gh_d = dpool.tile([nblk * 128, 8 * GC], bf16)
            # h^T archive: hist tiles stored verbatim per 8-step block
            hsd_d = dpool.tile([nblk * 128, 8 * KH * BL], bf16)

            # ---- persistent SBUF ----
            wh_sb = pp.tile([128, KH * 3 * H], bf16)
            why_sb = pp.tile([128, KH * O], bf16)
            id_sb = pp.tile([128, 128], bf16)
            idf_sb = pp.tile([128, 128], f32)
            bhh_sb = pp.tile([128, GC], bf16)
            bias_sb = pp.tile([128, 3 * H], f32)
            bhy_sb = pp.tile([128, O], f32)
            ones_sb = pp.tile([1, 128], f32)
            bxrow_sb = pp.tile([1, 3 * H], f32)
            bhyrow_sb = pp.tile([1, O], f32)
            h_sb = pp.tile([128, GC], f32)
            hist = pp.tile([128, 8 * KH * BL], bf16)   # h^T history, 8 slots
            wx_sb = pp.tile([128, KI * 3 * H], bf16)
            xT_sb = pp.tile([128, KI * n_steps * BL], bf16)

            for k in range(KH):
                nc.sync.dma_start(
                    wh_sb[:, k * 3 * H:(k + 1) * 3 * H], wh_d[k * 128:(k + 1) * 128, :]
                )
                nc.sync.dma_start(
                    why_sb[:, k * O:(k + 1) * O], why_d[k * 128:(k + 1) * 128, :]
                )
            for k in range(KI):
                nc.sync.dma_start(
                    wx_sb[:, k * 3 * H:(k + 1) * 3 * H], wx_d[k * 128:(k + 1) * 128, :]
                )
                nc.sync.dma_start(
                    xT_sb[:, k * n_steps * BL:(k + 1) * n_steps * BL],
                    xT_d[k * 128:(k + 1) * 128, :],
                )
            nc.sync.dma_start(id_sb[:], id_d[:, :])
            nc.sync.dma_start(idf_sb[:], idf_d[:, :])
            nc.sync.dma_start(bhh_sb[:], bhh_d[:, :])
            nc.sync.dma_start(ones_sb[:], ones_d[:, :])
            nc.sync.dma_start(bxrow_sb[:], bx_d[:, :])
            nc.sync.dma_start(bhyrow_sb[:], bhy_d[:, :])

            # broadcast bias rows across partitions via rank-1 matmuls
            for c in range(6):
                pb = psio.tile([128, 512], f32, tag="io")
                nc.tensor.matmul(
                    pb[:], ones_sb[0:1, :], bxrow_sb[0:1, c * 512:(c + 1) * 512],
                    start=True, stop=True,
                )
                nc.vector.tensor_copy(bias_sb[:, c * 512:(c + 1) * 512], pb[:])
            pb = psio.tile([128, O], f32, tag="io")
            nc.tensor.matmul(
                pb[:], ones_sb[0:1, :], bhyrow_sb[0:1, :], start=True, stop=True
            )
            nc.vector.tensor_copy(bhy_sb[:], pb[:])

            # ---- Phase A: input projections ----
            # dst views in (blk, s, p, n) order so the src AP needs no
            # partition permute: addr = (blk*128+p)*8n + s*n + j
            gzr_w = gzr_d[:].rearrange(
                "(blk p) (s n) -> blk s p n", p=128, s=8
            )
            gh_w = gh_d[:].rearrange(
                "(blk p) (s n) -> blk s p n", p=128, s=8
            )
            for m in range(n_iter):          # M-tile = 16 steps x 8 batch
                for c in range(6):           # 512-col chunks of the 3072 gate cols
                    pa = psio.tile([128, 512], f32, tag="io")
                    for k in range(KI):
                        nc.tensor.matmul(
                            pa[:],
                            xT_sb[:, k * n_steps * BL + m * 128:
                                  k * n_steps * BL + m * 128 + 128],
                            wx_sb[:, k * 3 * H + c * 512:k * 3 * H + c * 512 + 512],
                            start=(k == 0), stop=(k == KI - 1),
                        )
                    ev = ep.tile([128, 512], bf16, tag="ev")
                    nc.vector.scalar_tensor_tensor(
                        ev[:], pa[:], 0.0, bias_sb[:, c * 512:(c + 1) * 512],
                        ALU.bypass, ALU.add,
                    )
                    for t1 in range(2):
                        if c < 4:
                            nc.sync.dma_start(
                                gzr_w[2 * m + t1, :, 32 * c:32 * c + BL, :],
                                ev[t1 * 64:(t1 + 1) * 64, :],
                            )
                        else:
                            for gg in range(2):
                                g = 2 * (c - 4) + gg
                                nc.sync.dma_start(
                                    gh_w[2 * m + t1, :, 32 * g:32 * g + BL, :],
                                    ev[t1 * 64:(t1 + 1) * 64,
                                       gg * GC:(gg + 1) * GC],
                                )

            # ---- Phase B: recurrence ----
            nc.vector.memset(hist[:], 0)
            nc.vector.memset(h_sb[:], 0.0)

            def step(j, gzr_blk, gh_blk):
                """One GRU step j in [0,16). Reads h^T from hist slot
                (j+7)%8, writes slot j%8."""
                s = j % 8
                sprev = (j + 7) % 8
                gzr_t = gzr_blk[:, s * 512:(s + 1) * 512]
                gh_t = gh_blk[:, s * GC:(s + 1) * GC]

                pzr = pszr.tile([128, 512], f32)
                ph = psh.tile([128, GC], f32)
                ptr = pstr.tile([128, GC], f32)

                def kmm(out_ap, wcol):
                    for k in range(KH):
                        for g in range(NG):
                            nc.tensor.matmul(
                                out_ap(g),
                                hist[:, sprev * (KH * BL) + k * BL:
                                     sprev * (KH * BL) + (k + 1) * BL],
                                wh_sb[:, k * 3 * H + wcol(g):
                                      k * 3 * H + wcol(g) + GC],
                                start=False, stop=(k == KH - 1),
                                tile_position=(0, 32 * g),
                                skip_group_check=True,
                            )

                def eadd(out_ap, src_ap):
                    # K=8 contraction over the real rows only: copies the 8
                    # real batch rows of each group into psum and zero-fills
                    # the 24 garbage rows (avoids 0*NaN from uninitialized
                    # DRAM rows of the gate buffers).
                    for g in range(NG):
                        nc.tensor.matmul(
                            out_ap(g),
                            id_sb[32 * g:32 * g + BL, 32 * g:32 * g + 32],
                            src_ap[32 * g:32 * g + BL, :],
                            start=True, stop=False,
                            tile_position=(32 * g, 32 * g),
                            skip_group_check=True,
                        )

                # r gate first (its sigmoid gates the candidate path)
                eadd(lambda g: pzr[32 * g:32 * g + 32, 256:512], gzr_t[:, 256:512])
                kmm(lambda g: pzr[32 * g:32 * g + BL, 256:512],
                    lambda g: g * 512 + 256)
                # candidate pre-gemm (+bhh)
                eadd(lambda g: ph[32 * g:32 * g + 32, :], bhh_sb[:])
                kmm(lambda g: ph[32 * g:32 * g + BL, :], lambda g: 2048 + g * GC)
                # z gate
                eadd(lambda g: pzr[32 * g:32 * g + 32, 0:256], gzr_t[:, 0:256])
                kmm(lambda g: pzr[32 * g:32 * g + BL, 0:256], lambda g: g * 512)

                r_sb = tp.tile([128, GC], f32, tag="r")
                z_sb = tp.tile([128, GC], f32, tag="z")
                t3 = tp.tile([128, GC], f32, tag="t3")
                prh = tp.tile([128, GC], f32, tag="prh")
                hc = tp.tile([128, GC], f32, tag="hc")
                dd = tp.tile([128, GC], f32, tag="dd")
                ee = tp.tile([128, GC], f32, tag="ee")

                nc.scalar.activation(r_sb[:], pzr[:, 256:512], AF.Sigmoid)
                nc.vector.scalar_tensor_tensor(
                    t3[:], ph[:], 0.0, r_sb[:], ALU.bypass, ALU.mult
                )
                nc.vector.tensor_tensor(prh[:], t3[:], gh_t, ALU.add)
                nc.scalar.activation(hc[:], prh[:], AF.Tanh)
                nc.scalar.activation(z_sb[:], pzr[:, 0:256], AF.Sigmoid)
                nc.vector.tensor_tensor(dd[:], hc[:], h_sb[:], ALU.subtract)
                nc.vector.tensor_tensor(ee[:], z_sb[:], dd[:], ALU.mult)
                nc.vector.tensor_tensor(h_sb[:], h_sb[:], ee[:], ALU.add)

                # re-transpose h into hist slot s
                nc.tensor.transpose(ptr[:, 0:128], h_sb[:, 0:128], idf_sb[:])
                nc.tensor.transpose(ptr[:, 128:256], h_sb[:, 128:256], idf_sb[:])
                hist_v = hist[:].rearrange(
                    "p (ss g two b) -> p ss two g b", ss=8, g=4, two=2
                )
                for par in range(2):
                    nc.scalar.copy(
                        hist_v[:, s, par, :, :],
                        ptr[:, par * 128:(par + 1) * 128]
                        .rearrange("p (g c) -> p g c", g=4)[:, :, 0:BL],
                    )

            with tc.For_i(
                0, n_iter, 1,
                hint_engines=(mybir.EngineType.PE,),
            ) as it:
                gzrA = gpool.tile([128, 8 * 512], bf16, tag="gzrA")
                gzrB = gpool.tile([128, 8 * 512], bf16, tag="gzrB")
                ghA = gpool.tile([128, 8 * GC], bf16, tag="ghA")
                ghB = gpool.tile([128, 8 * GC], bf16, tag="ghB")
                nc.sync.dma_start(gzrA[:], gzr_d[ds(it * 256, 128), :])
                nc.sync.dma_start(ghA[:], gh_d[ds(it * 256, 128), :])
                nc.sync.dma_start(gzrB[:], gzr_d[ds(it * 256 + 128, 128), :])
                nc.sync.dma_start(ghB[:], gh_d[ds(it * 256 + 128, 128), :])
                for j in range(TUNROLL):
                    step(j, gzrA if j < 8 else gzrB, ghA if j < 8 else ghB)
                    if j % 8 == 7:
                        nc.scalar.dma_start(
                            hsd_d[ds((it * 2 + j // 8) * 128, 128), :],
                            hist[:],
                        )

            # ---- Phase C: output projection ----
            for m in range(n_iter):
                pc = psio.tile([128, O], f32, tag="io")
                st = cpool.tile([128, 2 * 8 * KH * BL], bf16, tag="cst")
                for half in range(2):
                    nc.sync.dma_start(
                        st[:, half * 512:(half + 1) * 512],
                        hsd_d[(2 * m + half) * 128:(2 * m + half + 1) * 128, :],
                    )
                st2 = cpool.tile([128, 2 * 8 * KH * BL], bf16, tag="cst2")
                nc.vector.tensor_copy(
                    st2[:].rearrange(
                        "p (k blk ss b) -> p k blk ss b", blk=2, ss=8, k=KH
                    ),
                    st[:].rearrange(
                        "p (blk ss k b) -> p k blk ss b", blk=2, ss=8, k=KH
                    ),
                )
                for k in range(KH):
                    nc.tensor.matmul(
                        pc[:], st2[:, k * 128:(k + 1) * 128],
                        why_sb[:, k * O:(k + 1) * O],
                        start=(k == 0), stop=(k == KH - 1),
                    )
                oc = ep.tile([128, O], f32, tag="oc")
                nc.vector.scalar_tensor_tensor(
                    oc[:], pc[:], 0.0, bhy_sb[:], ALU.bypass, ALU.add
                )
                nc.sync.dma_start(
                    out_d[m * TUNROLL:(m + 1) * TUNROLL, :, :],
                    oc[:],
                )

    if legalize:
        _legalize_multi_waits(nc)
    return nc


# ---------------- host side ----------------

_CACHE = {}
LAST_EXEC_NS = None


def _gate_col_perm():
    """Map the gate-col layout to canonical [z 1024 | r 1024 | h 1024]
    column ids: layout_col -> canonical_col."""
    cols = np.empty(3 * H, np.int64)
    for g in range(NG):
        for j in range(GC):
            cols[g * 512 + j] = g * GC + j               # z
            cols[g * 512 + 256 + j] = H + g * GC + j     # r
            cols[2048 + g * GC + j] = 2 * H + g * GC + j  # h
    return cols


def prep_shared(Wxz, bxz, Whz, bhz, Wxr, bxr, Whr, bhr, Wxh, bxh, Whh, bhh,
                Why, bhy):
    perm = _gate_col_perm()
    wx = np.concatenate([Wxz.T, Wxr.T, Wxh.T], axis=1)[:, perm]
    wh = np.concatenate([Whz.T, Whr.T, Whh.T], axis=1)[:, perm]
    bx = np.concatenate([bxz + bhz, bxr + bhr, bxh])[perm][None, :]
    bhh_bc = np.zeros((128, GC), np.float32)
    for g in range(NG):
        bhh_bc[32 * g:32 * (g + 1), :] = bhh[g * GC:(g + 1) * GC][None, :]
    return {
        "wxT": np.ascontiguousarray(wx).astype(BF),
        "whT": np.ascontiguousarray(wh).astype(BF),
        "whyT": np.ascontiguousarray(Why.T).astype(BF),
        "bxrow": bx.astype(np.float32),
        "bhyrow": bhy[None, :].astype(np.float32),
        "bhhbc": bhh_bc.astype(BF),
        "ident": np.eye(128, dtype=BF),
        "identf": np.eye(128, dtype=np.float32),
        "onesrow": np.ones((1, 128), np.float32),
    }


def kernel(x, Wxz, bxz, Whz, bhz, Wxr, bxr, Whr, bhr, Wxh, bxh, Whh, bhh,
           Why, bhy):
    global LAST_EXEC_NS
    from concourse.bass_utils import run_bass_kernel_spmd

    n_steps = x.shape[0]
    if "nc" not in _CACHE or _CACHE.get("n_steps") != n_steps:
        _CACHE["nc"] = build(n_steps)
        _CACHE["n_steps"] = n_steps
    nc = _CACHE["nc"]

    shared = prep_shared(Wxz, bxz, Whz, bhz, Wxr, bxr, Whr, bhr,
                         Wxh, bxh, Whh, bhh, Why, bhy)
    xbf = x.astype(BF)  # [S, B, I]
    in_maps = []
    for c in range(NCORES):
        xc = xbf[:, c * BL:(c + 1) * BL, :]                  # [S, 8, I]
        xT = np.ascontiguousarray(
            xc.reshape(n_steps * BL, I).T                     # [I, S*8]
        )
        in_maps.append({"xT": xT, **shared})

    t0 = time.time()
    res = run_bass_kernel_spmd(nc, in_maps, list(range(NCORES)))
    LAST_EXEC_NS = int((time.time() - t0) * 1e9)

    out = np.empty((n_steps, B, O), np.float32)
    for c in range(NCORES):
        out[:, c * BL:(c + 1) * BL, :] = res.results[c]["out"]
    return out


# revision 4
# speedup vs baseline: 1.0598x; 1.0598x over previous
"""GRU kernel for 8 NeuronCores — fully on-device.

Sharding: data-parallel over batch (8 batch rows per core), per the
sharding hint. Each core runs the whole model for its batch shard:

  Phase A: input projections  gates_x = x @ [Wxz|Wxr|Wxh]^T + biases
           (bf16 matmuls, fp32 psum, evacuated to DRAM in the
           "stacked" layout the recurrence consumes).
  Phase B: 512-step recurrence. The per-step GEMM h @ Wh*^T runs with
           the transposed hidden state as the 128x8 stationary operand
           and the weights as the bf16 moving operand, in 128x32
           column-tiled mode: 4 concurrent column-tiles each compute a
           256-wide chunk of the hidden dimension, so the 3*H*H weight
           stream costs ~6k cycles/step instead of ~24k.
           gates_x is folded into PSUM with shifted-identity matmuls,
           sigmoids/tanh read PSUM directly on ScalarE, the h-update
           runs on VectorE, and the new h is re-transposed with two PE
           transposes into an 8-slot static history buffer that serves
           both as next-step stationary operand and as the archive-DMA
           source (matmul stationaries cannot take register offsets).
  Phase C: output projection from the DRAM h^T archive.

Layouts
  stacked [128, n]: partition 32g+b (g=hidden col-group 0..3, b=batch
  row 0..7) holds hidden cols [256g, 256g+n) for batch row b.
  gate-col layout (3072): cols g*512+gate*256+j for z/r (gate 0/1),
  2048+g*256+j for the candidate gate.
  hist [128, 512] bf16: col s*64 + k*8 + b = h^T[128k+p, b] at slot
  s = t%8.
  gzr/gh DRAM: [(S/8)*128, 8*n] — row blk*128+p, col s*n+j.
  hsd archive [128, 8, S*8]: (p, k, t*8+b).
"""
import time

import numpy as np
import ml_dtypes

import concourse.bass as bass
import concourse.tile as tile
from concourse import mybir
from concourse.bass import ds

SEQ, B, I, H, O = 512, 64, 512, 1024, 512
NCORES = 8
BL = B // NCORES          # 8 batch rows per core
NG = 4                    # column groups
GC = H // NG              # 256 hidden cols per group
TUNROLL = 16
KH = H // 128             # 8 contraction chunks for H
KI = I // 128             # 4 contraction chunks for I

bf16 = mybir.dt.bfloat16
f32 = mybir.dt.float32
AF = mybir.ActivationFunctionType
ALU = mybir.AluOpType

BF = ml_dtypes.bfloat16


def _legalize_multi_waits(nc):
    """This container's walrus encodes at most ONE sync-wait per TPB
    instruction and dies with "Too many sync wait commands" on anything
    carrying more (e.g. the Tile end-of-kernel drain). Hoist all-but-one
    wait of every instruction onto standalone sequencer EventSemaphore
    instructions placed immediately before it in the same engine stream."""
    n_multi_update = 0
    uid = [0]
    for fn in nc.m.functions:
        for bb in fn.blocks:
            insts = list(bb.instructions)
            out = []
            changed = False
            for inst in insts:
                si = inst.sync_info
                if si is not None and len(si.on_wait) > 1:
                    waits = list(si.on_wait)
                    for w in waits[:-1]:
                        uid[0] += 1
                        ev = mybir.InstEventSemaphore(
                            name=f"legalw-{uid[0]}", ins=[], outs=[]
                        )
                        ev.engine = inst.engine
                        ev.sync_info = mybir.SyncInfo(on_wait=[w], on_update=[])
                        out.append(ev)
                    inst.sync_info = mybir.SyncInfo(
                        on_wait=[waits[-1]], on_update=list(si.on_update)
                    )
                    changed = True
                if si is not None and len(si.on_update) > 1:
                    n_multi_update += 1
                out.append(inst)
            if changed:
                bb.instructions = out
    assert n_multi_update == 0


def build(n_steps=SEQ, legalize=True):
    n_iter = n_steps // TUNROLL
    assert n_iter * TUNROLL == n_steps
    nblk = n_steps // 8
    nc = bass.Bass()

    xT_d = nc.dram_tensor("xT", [I, n_steps * BL], bf16, kind="ExternalInput")
    wx_d = nc.dram_tensor("wxT", [I, 3 * H], bf16, kind="ExternalInput")
    wh_d = nc.dram_tensor("whT", [H, 3 * H], bf16, kind="ExternalInput")
    why_d = nc.dram_tensor("whyT", [H, O], bf16, kind="ExternalInput")
    bx_d = nc.dram_tensor("bxrow", [1, 3 * H], f32, kind="ExternalInput")
    bhy_d = nc.dram_tensor("bhyrow", [1, O], f32, kind="ExternalInput")
    bhh_d = nc.dram_tensor("bhhbc", [128, GC], bf16, kind="ExternalInput")
    id_d = nc.dram_tensor("ident", [128, 128], bf16, kind="ExternalInput")
    idf_d = nc.dram_tensor("identf", [128, 128], f32, kind="ExternalInput")
    ones_d = nc.dram_tensor("onesrow", [1, 128], f32, kind="ExternalInput")
    out_d = nc.dram_tensor("out", [n_steps, BL, O], f32, kind="ExternalOutput")

    with tile.TileContext(nc) as tc:
        with (
            tc.tile_pool(name="dram", bufs=1, space="DRAM") as dpool,
            tc.tile_pool(name="persist", bufs=1) as pp,
            tc.tile_pool(name="gin", bufs=1) as gpool,
            tc.tile_pool(name="tail", bufs=2) as tp,
            tc.tile_pool(name="evac", bufs=3) as ep,
            tc.tile_pool(name="cstat", bufs=3) as cpool,
            tc.tile_pool(name="pszr", bufs=2, space="PSUM") as pszr,
            tc.tile_pool(name="psh", bufs=2, space="PSUM") as psh,
            tc.tile_pool(name="pstr", bufs=2, space="PSUM") as pstr,
            tc.tile_pool(name="psio", bufs=2, space="PSUM") as psio,
        ):
            gzr_d = dpool.tile([nblk * 128, 8 * 512], bf16)
            gh_d = dpool.tile([nblk * 128, 8 * GC], bf16)
            # h^T archive: hist tiles stored verbatim per 8-step block
            hsd_d = dpool.tile([nblk * 128, 8 * KH * BL], bf16)

            # ---- persistent SBUF ----
            wh_sb = pp.tile([128, KH * 3 * H], bf16)
            why_sb = pp.tile([128, KH * O], bf16)
            id_sb = pp.tile([128, 128], bf16)
            idf_sb = pp.tile([128, 128], f32)
            bhh_sb = pp.tile([128, GC], bf16)
            bias_sb = pp.tile([128, 3 * H], f32)
            bhy_sb = pp.tile([128, O], f32)
            ones_sb = pp.tile([1, 128], f32)
            bxrow_sb = pp.tile([1, 3 * H], f32)
            bhyrow_sb = pp.tile([1, O], f32)
            h_sb = pp.tile([128, GC], f32)
            hist = pp.tile([128, 8 * KH * BL], bf16)   # h^T history, 8 slots
            wx_sb = pp.tile([128, KI * 3 * H], bf16)
            xT_sb = pp.tile([128, KI * n_steps * BL], bf16)

            for k in range(KH):
                nc.sync.dma_start(
                    wh_sb[:, k * 3 * H:(k + 1) * 3 * H], wh_d[k * 128:(k + 1) * 128, :]
                )
                nc.sync.dma_start(
                    why_sb[:, k * O:(k + 1) * O], why_d[k * 128:(k + 1) * 128, :]
                )
            for k in range(KI):
                nc.sync.dma_start(
                    wx_sb[:, k * 3 * H:(k + 1) * 3 * H], wx_d[k * 128:(k + 1) * 128, :]
                )
                nc.sync.dma_start(
                    xT_sb[:, k * n_steps * BL:(k + 1) * n_steps * BL],
                    xT_d[k * 128:(k + 1) * 128, :],
                )
            nc.sync.dma_start(id_sb[:], id_d[:, :])
            nc.sync.dma_start(idf_sb[:], idf_d[:, :])
            nc.sync.dma_start(bhh_sb[:], bhh_d[:, :])
            nc.sync.dma_start(ones_sb[:], ones_d[:, :])
            nc.sync.dma_start(bxrow_sb[:], bx_d[:, :])
            nc.sync.dma_start(bhyrow_sb[:], bhy_d[:, :])

            # broadcast bias rows across partitions via rank-1 matmuls
            for c in range(6):
                pb = psio.tile([128, 512], f32, tag="io")
                nc.tensor.matmul(
                    pb[:], ones_sb[0:1, :], bxrow_sb[0:1, c * 512:(c + 1) * 512],
                    start=True, stop=True,
                )
                nc.vector.tensor_copy(bias_sb[:, c * 512:(c + 1) * 512], pb[:])
            pb = psio.tile([128, O], f32, tag="io")
            nc.tensor.matmul(
                pb[:], ones_sb[0:1, :], bhyrow_sb[0:1, :], start=True, stop=True
            )
            nc.vector.tensor_copy(bhy_sb[:], pb[:])

            # ---- Phase A: input projections ----
            # dst views in (blk, s, p, n) order so the src AP needs no
            # partition permute: addr = (blk*128+p)*8n + s*n + j
            gzr_w = gzr_d[:].rearrange(
                "(blk p) (s n) -> blk s p n", p=128, s=8
            )
            gh_w = gh_d[:].rearrange(
                "(blk p) (s n) -> blk s p n", p=128, s=8
            )
            for m in range(n_iter):          # M-tile = 16 steps x 8 batch
                for c in range(6):           # 512-col chunks of the 3072 gate cols
                    pa = psio.tile([128, 512], f32, tag="io")
                    for k in range(KI):
                        nc.tensor.matmul(
                            pa[:],
                            xT_sb[:, k * n_steps * BL + m * 128:
                                  k * n_steps * BL + m * 128 + 128],
                            wx_sb[:, k * 3 * H + c * 512:k * 3 * H + c * 512 + 512],
                            start=(k == 0), stop=(k == KI - 1),
                        )
                    ev = ep.tile([128, 512], bf16, tag="ev")
                    nc.vector.scalar_tensor_tensor(
                        ev[:], pa[:], 0.0, bias_sb[:, c * 512:(c + 1) * 512],
                        ALU.bypass, ALU.add,
                    )
                    for t1 in range(2):
                        if c < 4:
                            nc.sync.dma_start(
                                gzr_w[2 * m + t1, :, 32 * c:32 * c + BL, :],
                                ev[t1 * 64:(t1 + 1) * 64, :],
                            )
                        else:
                            for gg in range(2):
                                g = 2 * (c - 4) + gg
                                nc.sync.dma_start(
                                    gh_w[2 * m + t1, :, 32 * g:32 * g + BL, :],
                                    ev[t1 * 64:(t1 + 1) * 64,
                                       gg * GC:(gg + 1) * GC],
                                )

            # ---- Phase B: recurrence ----
            nc.vector.memset(hist[:], 0)
            nc.vector.memset(h_sb[:], 0.0)

            def step(j, gzr_blk, gh_blk):
                """One GRU step j in [0,16). Reads h^T from hist slot
                (j+7)%8, writes slot j%8."""
                s = j % 8
                sprev = (j + 7) % 8
                gzr_t = gzr_blk[:, s * 512:(s + 1) * 512]
                gh_t = gh_blk[:, s * GC:(s + 1) * GC]

                pzr = pszr.tile([128, 512], f32)
                ph = psh.tile([128, GC], f32)
                ptr = pstr.tile([128, GC], f32)

                def kmm(out_ap, wcol):
                    for k in range(KH):
                        for g in range(NG):
                            nc.tensor.matmul(
                                out_ap(g),
                                hist[:, sprev * (KH * BL) + k * BL:
                                     sprev * (KH * BL) + (k + 1) * BL],
                                wh_sb[:, k * 3 * H + wcol(g):
                                      k * 3 * H + wcol(g) + GC],
                                start=False, stop=(k == KH - 1),
                                tile_position=(0, 32 * g),
                                skip_group_check=True,
                            )

                def eadd(out_ap, src_ap):
                    # K=8 contraction over the real rows only: copies the 8
                    # real batch rows of each group into psum and zero-fills
                    # the 24 garbage rows (avoids 0*NaN from uninitialized
                    # DRAM rows of the gate buffers).
                    for g in range(NG):
                        nc.tensor.matmul(
                            out_ap(g),
                            id_sb[32 * g:32 * g + BL, 32 * g:32 * g + 32],
                            src_ap[32 * g:32 * g + BL, :],
                            start=True, stop=False,
                            tile_position=(32 * g, 32 * g),
                            skip_group_check=True,
                        )

                # r gate first (its sigmoid gates the candidate path)
                eadd(lambda g: pzr[32 * g:32 * g + 32, 256:512], gzr_t[:, 256:512])
                kmm(lambda g: pzr[32 * g:32 * g + BL, 256:512],
                    lambda g: g * 512 + 256)
                # candidate pre-gemm (+bhh)
                eadd(lambda g: ph[32 * g:32 * g + 32, :], bhh_sb[:])
                kmm(lambda g: ph[32 * g:32 * g + BL, :], lambda g: 2048 + g * GC)
                # z gate
                eadd(lambda g: pzr[32 * g:32 * g + 32, 0:256], gzr_t[:, 0:256])
                kmm(lambda g: pzr[32 * g:32 * g + BL, 0:256], lambda g: g * 512)

                r_sb = tp.tile([128, GC], f32, tag="r")
                z_sb = tp.tile([128, GC], f32, tag="z")
                t3 = tp.tile([128, GC], f32, tag="t3")
                prh = tp.tile([128, GC], f32, tag="prh")
                hc = tp.tile([128, GC], f32, tag="hc")
                dd = tp.tile([128, GC], f32, tag="dd")
                ee = tp.tile([128, GC], f32, tag="ee")

                nc.scalar.activation(r_sb[:], pzr[:, 256:512], AF.Sigmoid)
                nc.vector.scalar_tensor_tensor(
                    t3[:], ph[:], 0.0, r_sb[:], ALU.bypass, ALU.mult
                )
                nc.vector.tensor_tensor(prh[:], t3[:], gh_t, ALU.add)
                nc.scalar.activation(hc[:], prh[:], AF.Tanh)
                nc.scalar.activation(z_sb[:], pzr[:, 0:256], AF.Sigmoid)
                nc.vector.tensor_tensor(dd[:], hc[:], h_sb[:], ALU.subtract)
                nc.vector.tensor_tensor(ee[:], z_sb[:], dd[:], ALU.mult)
                nc.vector.tensor_tensor(h_sb[:], h_sb[:], ee[:], ALU.add)

                # re-transpose h into hist slot s
                nc.tensor.transpose(ptr[:, 0:128], h_sb[:, 0:128], idf_sb[:])
                nc.tensor.transpose(ptr[:, 128:256], h_sb[:, 128:256], idf_sb[:])
                hist_v = hist[:].rearrange(
                    "p (ss g two b) -> p ss two g b", ss=8, g=4, two=2
                )
                for par in range(2):
                    nc.scalar.copy(
                        hist_v[:, s, par, :, :],
                        ptr[:, par * 128:(par + 1) * 128]
                        .rearrange("p (g c) -> p g c", g=4)[:, :, 0:BL],
                    )

            with tc.For_i(
                0, n_iter, 1,
                hint_engines=(mybir.EngineType.PE,),
            ) as it:
                gzrA = gpool.tile([128, 8 * 512], bf16, tag="gzrA")
                gzrB = gpool.tile([128, 8 * 512], bf16, tag="gzrB")
                ghA = gpool.tile([128, 8 * GC], bf16, tag="ghA")
                ghB = gpool.tile([128, 8 * GC], bf16, tag="ghB")
                nc.sync.dma_start(gzrA[:], gzr_d[ds(it * 256, 128), :])
                nc.sync.dma_start(ghA[:], gh_d[ds(it * 256, 128), :])
                nc.sync.dma_start(gzrB[:], gzr_d[ds(it * 256 + 128, 128), :])
                nc.sync.dma_start(ghB[:], gh_d[ds(it * 256 + 128, 128), :])
                for j in range(TUNROLL):
                    step(j, gzrA if j < 8 else gzrB, ghA if j < 8 else ghB)
                    if j % 8 == 7:
                        nc.scalar.dma_start(
                            hsd_d[ds((it * 2 + j // 8) * 128, 128), :],
                            hist[:],
                        )

            # ---- Phase C: output projection ----
            for m in range(n_iter):
                pc = psio.tile([128, O], f32, tag="io")
                st = cpool.tile([128, 2 * 8 * KH * BL], bf16, tag="cst")
                for half in range(2):
                    nc.sync.dma_start(
                        st[:, half * 512:(half + 1) * 512],
                        hsd_d[(2 * m + half) * 128:(2 * m + half + 1) * 128, :],
                    )
                st2 = cpool.tile([128, 2 * 8 * KH * BL], bf16, tag="cst2")
                nc.vector.tensor_copy(
                    st2[:].rearrange(
                        "p (k blk ss b) -> p k blk ss b", blk=2, ss=8, k=KH
                    ),
                    st[:].rearrange(
                        "p (blk ss k b) -> p k blk ss b", blk=2, ss=8, k=KH
                    ),
                )
                for k in range(KH):
                    nc.tensor.matmul(
                        pc[:], st2[:, k * 128:(k + 1) * 128],
                        why_sb[:, k * O:(k + 1) * O],
                        start=(k == 0), stop=(k == KH - 1),
                    )
                oc = ep.tile([128, O], f32, tag="oc")
                nc.vector.scalar_tensor_tensor(
                    oc[:], pc[:], 0.0, bhy_sb[:], ALU.bypass, ALU.add
                )
                nc.sync.dma_start(
                    out_d[m * TUNROLL:(m + 1) * TUNROLL, :, :],
                    oc[:],
                )

    if legalize:
        _legalize_multi_waits(nc)
    return nc


# ---------------- host side ----------------

_CACHE = {}
LAST_EXEC_NS = None


def _gate_col_perm():
    """Map the gate-col layout to canonical [z 1024 | r 1024 | h 1024]
    column ids: layout_col -> canonical_col."""
    cols = np.empty(3 * H, np.int64)
    for g in range(NG):
        for j in range(GC):
            cols[g * 512 + j] = g * GC + j               # z
            cols[g * 512 + 256 + j] = H + g * GC + j     # r
            cols[2048 + g * GC + j] = 2 * H + g * GC + j  # h
    return cols


def prep_shared(Wxz, bxz, Whz, bhz, Wxr, bxr, Whr, bhr, Wxh, bxh, Whh, bhh,
                Why, bhy):
    perm = _gate_col_perm()
    wx = np.concatenate([Wxz.T, Wxr.T, Wxh.T], axis=1)[:, perm]
    wh = np.concatenate([Whz.T, Whr.T, Whh.T], axis=1)[:, perm]
    bx = np.concatenate([bxz + bhz, bxr + bhr, bxh])[perm][None, :]
    bhh_bc = np.zeros((128, GC), np.float32)
    for g in range(NG):
        bhh_bc[32 * g:32 * (g + 1), :] = bhh[g * GC:(g + 1) * GC][None, :]
    return {
        "wxT": np.ascontiguousarray(wx).astype(BF),
        "whT": np.ascontiguousarray(wh).astype(BF),
        "whyT": np.ascontiguousarray(Why.T).astype(BF),
        "bxrow": bx.astype(np.float32),
        "bhyrow": bhy[None, :].astype(np.float32),
        "bhhbc": bhh_bc.astype(BF),
        "ident": np.eye(128, dtype=BF),
        "identf": np.eye(128, dtype=np.float32),
        "onesrow": np.ones((1, 128), np.float32),
    }


def _make_runner(nc, n_cores):
    """Cached PJRT runner — mirrors bass2jax.run_bass_via_pjrt but (a)
    builds the jitted function once per process and (b) passes the
    weight-like inputs replicated (P(None)) so only one copy crosses
    the host->device link instead of eight."""
    import jax
    import numpy as _np
    from jax.sharding import Mesh, PartitionSpec
    from jax.experimental.shard_map import shard_map
    from concourse import bass2jax, mybir as _mb

    bass2jax.install_neuronx_cc_hook()
    partition_name = (
        nc.partition_id_tensor.name if nc.partition_id_tensor else None
    )
    in_names, out_names, out_avals, zero_outs = [], [], [], []
    for alloc in nc.m.functions[0].allocations:
        if not isinstance(alloc, _mb.MemoryLocationSet):
            continue
        name = alloc.memorylocations[0].name
        if alloc.kind == "ExternalInput":
            if name != partition_name:
                in_names.append(name)
        elif alloc.kind == "ExternalOutput":
            shape = tuple(alloc.tensor_shape)
            dtype = _mb.dt.np(alloc.dtype)
            out_names.append(name)
            out_avals.append(jax.core.ShapedArray(shape, dtype))
            zero_outs.append(_np.zeros(shape, dtype))
    n_params = len(in_names)
    n_outs = len(out_avals)
    all_names = in_names + out_names
    if partition_name is not None:
        all_names.append(partition_name)
    sharded_in = {"xT"}

    def _body(*args):
        operands = list(args)
        if partition_name is not None:
            operands.append(bass2jax.partition_id_tensor())
        outs = bass2jax._bass_exec_p.bind(
            *operands,
            out_avals=tuple(out_avals),
            in_names=tuple(all_names),
            out_names=tuple(out_names),
            lowering_input_output_aliases=(),
            sim_require_finite=True,
            sim_require_nnan=True,
            nc=nc,
        )
        return tuple(outs)

    devices = jax.devices()[:n_cores]
    mesh = Mesh(_np.asarray(devices), ("core",))
    in_specs = tuple(
        PartitionSpec("core") if nm in sharded_in else PartitionSpec()
        for nm in in_names
    ) + (PartitionSpec("core"),) * n_outs
    out_specs = (PartitionSpec("core"),) * n_outs
    donate = tuple(range(n_params, n_params + n_outs))
    fn = jax.jit(
        shard_map(
            _body, mesh=mesh, in_specs=in_specs, out_specs=out_specs,
            check_rep=False,
        ),
        donate_argnums=donate,
        keep_unused=True,
    )

    def run(per_core_xT, shared_map):
        ins = []
        for nm in in_names:
            if nm in sharded_in:
                ins.append(_np.concatenate(per_core_xT, axis=0))
            else:
                ins.append(_np.asarray(shared_map[nm]))
        zouts = [
            _np.zeros((n_cores * z.shape[0], *z.shape[1:]), z.dtype)
            for z in zero_outs
        ]
        outs = fn(*ins, *zouts)
        return {
            nm: _np.asarray(outs[i]).reshape(n_cores, *out_avals[i].shape)
            for i, nm in enumerate(out_names)
        }

    return run


def kernel(x, Wxz, bxz, Whz, bhz, Wxr, bxr, Whr, bhr, Wxh, bxh, Whh, bhh,
           Why, bhy):
    global LAST_EXEC_NS
    n_steps = x.shape[0]
    if _CACHE.get("n_steps") != n_steps:
        nc = build(n_steps)
        _CACHE["run"] = _make_runner(nc, NCORES)
        _CACHE["n_steps"] = n_steps
    run = _CACHE["run"]

    shared = prep_shared(Wxz, bxz, Whz, bhz, Wxr, bxr, Whr, bhr,
                         Wxh, bxh, Whh, bhh, Why, bhy)
    xbf = x.astype(BF)  # [S, B, I]
    per_core_xT = []
    for c in range(NCORES):
        xc = xbf[:, c * BL:(c + 1) * BL, :]                  # [S, 8, I]
        per_core_xT.append(np.ascontiguousarray(
            xc.reshape(n_steps * BL, I).T                     # [I, S*8]
        ))

    t0 = time.time()
    res = run(per_core_xT, shared)
    LAST_EXEC_NS = int((time.time() - t0) * 1e9)

    out = np.empty((n_steps, B, O), np.float32)
    for c in range(NCORES):
        out[:, c * BL:(c + 1) * BL, :] = res["out"][c]
    return out
